# revision 15
# baseline (speedup 1.0000x reference)
"""Trainium2 Bass kernel for nn_CF_spikes (integrate-and-fire SNN, 784-128-64-10, 16 steps).

Strategy (pure data parallel over batch, 8 cores x 4096 rows):
  - Encoder cumulative spike count C_t = floor(t*f) computed on-chip:
      ACT:  y = f*t - 0.5            (one op, exact fp32)
      DVE:  C = (y + 2^23*1.5) - 2^23*1.5   (round-to-int trick, chained tensor_scalar)
  - Layer 1 membrane per step in fresh PSUM:
      psum1 = C_t @ W1r^T  - 0.5*cumsig1  - t*u_corr
    where W1r = fp32r(W1) and u_corr = fp32r(f) @ fp32r(W1r-W1)^T corrects the
    fp32r weight rounding (dominant error term), computed once per block.
    Spikes as sigma = +-1 via ACT Sign with per-partition folded thresholds.
  - Layers 2+3 fused into one persistent PSUM region [84, N]:
      cols 0:64 v2-state, 64:74 v3-state, 74:84 spike counts; matmuls accumulate
      sigma1 @ [W2^T/2|0|0] (+lo part) and [sigma2;sigma3] @ [-I/2|W3^T/2|0; 0|-I/2|I/2]
      (+lo), with biases/resets/rate-decode folded into per-partition thresholds.
  - Output: counts/16*out_scale via one ACT Identity with per-partition scale+bias.

All matmuls in float32r (1 cyc/row): C, sigma, cumsig are small ints (exact);
weight rounding handled by the correction/lo-split terms (rel err ~5e-3).
"""
import numpy as np
from contextlib import ExitStack

import concourse.bacc as bacc
import concourse.mybir as mybir
import concourse.tile as tile
from concourse.bass_utils import run_bass_kernel_spmd

f32 = mybir.dt.float32
f32r = mybir.dt.float32r
ALU = mybir.AluOpType
AF = mybir.ActivationFunctionType

DEBUG_DUMP = False
DUR = 16
B, IN, H1, H2, OUT = 32768, 784, 128, 64, 10
NCORES = 8
B_CORE = B // NCORES          # 4096
BLK = 512
NBLK = B_CORE // BLK          # 8
PCH = 112                     # partition rows per feature chunk
NCH = IN // PCH               # 7
MAGIC = 12582912.0            # 1.5 * 2^23
NT23 = H2 + OUT               # 74 rows of sign23
MOUT = H2 + 2 * OUT           # 84 psum23 cols (v2 | v3 | counts)


def fp32r_round(x):
    u = np.ascontiguousarray(np.asarray(x, np.float32)).view(np.uint32)
    r = ((u.astype(np.uint64) + 0x800) & 0xFFFFF000).astype(np.uint32)
    return r.view(np.float32).reshape(np.asarray(x).shape)


def _build_program():
    nc = bacc.Bacc(None, target_bir_lowering=False)
    ft_e = nc.declare_dram_parameter("ft", [PCH, NCH, B_CORE], f32, isOutput=False)
    w1t_e = nc.declare_dram_parameter("w1t", [PCH, NCH, H1], f32r, isOutput=False)
    dw1t_e = nc.declare_dram_parameter("dw1t", [PCH, NCH, H1], f32r, isOutput=False)
    nhi_e = nc.declare_dram_parameter("nhi", [H1, H1], f32r, isOutput=False)
    itneg_e = nc.declare_dram_parameter("itneg", [H1, DUR, H1], f32r, isOutput=False)
    lhsA_e = nc.declare_dram_parameter("lhsA", [H1, MOUT], f32r, isOutput=False)
    lhsAlo_e = nc.declare_dram_parameter("lhsAlo", [H1, MOUT], f32r, isOutput=False)
    lhsB_e = nc.declare_dram_parameter("lhsB", [NT23, MOUT], f32r, isOutput=False)
    lhsBlo_e = nc.declare_dram_parameter("lhsBlo", [NT23, MOUT], f32r, isOutput=False)
    nthr1_e = nc.declare_dram_parameter("nthr1", [H1, DUR], f32, isOutput=False)
    nthr23_e = nc.declare_dram_parameter("nthr23", [NT23, DUR + 1], f32, isOutput=False)
    osc_e = nc.declare_dram_parameter("osc", [2 * OUT, 1], f32, isOutput=False)
    obi_e = nc.declare_dram_parameter("obi", [2 * OUT, 1], f32, isOutput=False)
    out_e = nc.declare_dram_parameter("outT", [OUT, B_CORE], f32, isOutput=True)
    if DEBUG_DUMP:
        dbg_sg1_e = nc.declare_dram_parameter("dbg_sg1", [H1, DUR, B_CORE], f32, isOutput=True)
        dbg_uc_e = nc.declare_dram_parameter("dbg_uc", [H1, B_CORE], f32, isOutput=True)
        dbg_c_e = nc.declare_dram_parameter("dbg_c", [PCH, NCH, B_CORE], f32, isOutput=True)
        dbg_s23_e = nc.declare_dram_parameter("dbg_s23", [NT23, DUR + 1, B_CORE], f32, isOutput=True)
        dbg_ps_e = nc.declare_dram_parameter("dbg_ps", [NT23, 3, B_CORE], f32, isOutput=True)
        dbg_pre_e = nc.declare_dram_parameter("dbg_pre", [NT23, B_CORE], f32, isOutput=True)
        dbg_sg1b_e = nc.declare_dram_parameter("dbg_sg1b", [H1, 4, B_CORE], f32, isOutput=True)
        dbg_ps1_e = nc.declare_dram_parameter("dbg_ps1", [H1, 4, B_CORE], f32, isOutput=True)
        dbg_y2_e = nc.declare_dram_parameter("dbg_y2", [PCH, NCH, B_CORE], f32, isOutput=True)
        dbg_c2_e = nc.declare_dram_parameter("dbg_c2", [PCH, NCH, B_CORE], f32, isOutput=True)

    with tile.TileContext(nc) as tc, ExitStack() as ctx:
        const = ctx.enter_context(tc.tile_pool(name="const", bufs=1))
        fbuf = ctx.enter_context(tc.tile_pool(name="fbuf", bufs=3))
        frbuf = ctx.enter_context(tc.tile_pool(name="frbuf", bufs=1))
        ybuf = ctx.enter_context(tc.tile_pool(name="ybuf", bufs=3))
        cbuf = ctx.enter_context(tc.tile_pool(name="cbuf", bufs=3))
        state = ctx.enter_context(tc.tile_pool(name="state", bufs=3))
        sgbuf = ctx.enter_context(tc.tile_pool(name="sgbuf", bufs=3))
        s23buf = ctx.enter_context(tc.tile_pool(name="s23buf", bufs=4))
        obuf = ctx.enter_context(tc.tile_pool(name="obuf", bufs=2))
        p1 = ctx.enter_context(tc.tile_pool(name="p1", bufs=3, space="PSUM"))
        p23 = ctx.enter_context(tc.tile_pool(name="p23", bufs=3, space="PSUM"))
        pu = ctx.enter_context(tc.tile_pool(name="pu", bufs=1, space="PSUM"))

        w1t = const.tile([PCH, NCH, H1], f32r)
        dw1t = const.tile([PCH, NCH, H1], f32r)
        nhi = const.tile([H1, H1], f32r)
        itneg = const.tile([H1, DUR, H1], f32r)
        lhsA = const.tile([H1, MOUT], f32r)
        lhsAlo = const.tile([H1, MOUT], f32r)
        lhsB = const.tile([NT23, MOUT], f32r)
        lhsBlo = const.tile([NT23, MOUT], f32r)
        nthr1 = const.tile([H1, DUR], f32)
        nthr23 = const.tile([NT23, DUR + 1], f32)
        osc = const.tile([2 * OUT, 1], f32)
        obi = const.tile([2 * OUT, 1], f32)
        for t_, e_ in [(w1t, w1t_e), (dw1t, dw1t_e), (nhi, nhi_e), (itneg, itneg_e),
                       (lhsA, lhsA_e), (lhsAlo, lhsAlo_e), (lhsB, lhsB_e),
                       (lhsBlo, lhsBlo_e), (nthr1, nthr1_e), (nthr23, nthr23_e),
                       (osc, osc_e), (obi, obi_e)]:
            nc.sync.dma_start(out=t_, in_=e_[:])

        def block_setup(b):
            """DMA f-block, compute u_corr, init states. Returns dict of tiles."""
            sl = slice(b * BLK, (b + 1) * BLK)
            fb = fbuf.tile([PCH, NCH * BLK], f32, tag="fb")
            nc.sync.dma_start(out=fb.rearrange("p (g n) -> p g n", g=NCH), in_=ft_e[:, :, sl])
            fbr = frbuf.tile([PCH, NCH * BLK], f32r, tag="fbr")
            nc.vector.tensor_copy(fbr, fb)
            psu = pu.tile([H1, BLK], f32, tag="psu")
            for g in range(NCH):
                nc.tensor.matmul(psu, dw1t[:, g, :], fbr[:, g * BLK:(g + 1) * BLK],
                                 start=(g == 0), stop=(g == NCH - 1))
            ucorr = state.tile([H1, BLK], f32r, tag="ucorr")
            nc.vector.tensor_copy(ucorr, psu)
            if DEBUG_DUMP:
                nc.sync.dma_start(out=dbg_uc_e[:, b * BLK:(b + 1) * BLK], in_=ucorr.bitcast(f32))
            cumsg = state.tile([H1, BLK], f32r, tag="cumsg")
            nc.vector.memset(cumsg.bitcast(f32), 0.0)
            s23 = s23buf.tile([NT23, BLK], f32r, tag="s23")
            nc.vector.memset(s23.bitcast(f32), -1.0)
            ps23 = p23.tile([MOUT, BLK], f32, tag="ps23")
            return dict(fb=fb, ucorr=ucorr, cumsg=cumsg, s23=s23, ps23=ps23, first=True, bidx=b)

        def block_step(st, t):
            """One timestep t (1-based) for a block."""
            fb, ucorr, cumsg, s23, ps23 = st["fb"], st["ucorr"], st["cumsg"], st["s23"], st["ps23"]
            if DEBUG_DUMP and t == 2:
                pre = obuf.tile([NT23, BLK], f32, tag="pscp")
                nc.vector.tensor_copy(pre, ps23[0:NT23, :])
                b_ = st["bidx"]
                nc.sync.dma_start(out=dbg_pre_e[:, b_ * BLK:(b_ + 1) * BLK], in_=pre)
            y = ybuf.tile([PCH, NCH * BLK], f32, tag="y")
            nc.scalar.activation(y, fb, AF.Copy, bias=-0.5, scale=float(t))
            c = cbuf.tile([PCH, NCH * BLK], f32r, tag="c")
            nc.vector.tensor_scalar(out=c, in0=y, scalar1=MAGIC, scalar2=-MAGIC,
                                    op0=ALU.add, op1=ALU.add)
            ps1 = p1.tile([H1, BLK], f32, tag="ps1")
            for g in range(NCH):
                nc.tensor.matmul(ps1, w1t[:, g, :], c[:, g * BLK:(g + 1) * BLK],
                                 start=(g == 0), stop=False)
            nc.tensor.matmul(ps1, nhi, cumsg, start=False, stop=False)
            nc.tensor.matmul(ps1, itneg[:, t - 1, :], ucorr, start=False, stop=True)
            sg1 = sgbuf.tile([H1, BLK], f32r, tag="sg1")
            nc.scalar.activation(sg1, ps1, AF.Sign, bias=nthr1[:, t - 1:t], scale=1.0)
            if DEBUG_DUMP:
                b_ = st["bidx"]
                nc.sync.dma_start(out=dbg_sg1_e[:, t - 1, b_ * BLK:(b_ + 1) * BLK], in_=sg1.bitcast(f32))
                if t <= 4:
                    nc.sync.dma_start(out=dbg_sg1b_e[:, t - 1, b_ * BLK:(b_ + 1) * BLK], in_=sg1.bitcast(f32))
                    pcp1 = obuf.tile([H1, BLK], f32, tag="pcp1")
                    nc.vector.tensor_copy(pcp1, ps1)
                    nc.sync.dma_start(out=dbg_ps1_e[:, t - 1, b_ * BLK:(b_ + 1) * BLK], in_=pcp1)
                if t == 1:
                    nc.sync.dma_start(out=dbg_c_e[:, :, b_ * BLK:(b_ + 1) * BLK], in_=c.bitcast(f32).rearrange("p (g n) -> p g n", g=NCH))
                if t == 2:
                    nc.sync.dma_start(out=dbg_y2_e[:, :, b_ * BLK:(b_ + 1) * BLK], in_=y.rearrange("p (g n) -> p g n", g=NCH))
                    nc.sync.dma_start(out=dbg_c2_e[:, :, b_ * BLK:(b_ + 1) * BLK], in_=c.bitcast(f32).rearrange("p (g n) -> p g n", g=NCH))
            nc.vector.tensor_tensor(cumsg, cumsg, sg1, ALU.add)
            first = st["first"]
            nc.tensor.matmul(ps23, lhsA, sg1, start=first, stop=False,
                             skip_group_check=True)
            nc.tensor.matmul(ps23, lhsAlo, sg1, start=False, stop=False,
                             skip_group_check=True)
            nc.tensor.matmul(ps23, lhsB, s23, start=False, stop=False,
                             skip_group_check=True)
            nc.tensor.matmul(ps23, lhsBlo, s23, start=False, stop=False,
                             skip_group_check=True)
            st["first"] = False
            if DEBUG_DUMP and t <= 3:
                pscp = obuf.tile([NT23, BLK], f32, tag="pscp")
                nc.vector.tensor_copy(pscp, ps23[0:NT23, :])
                b_ = st["bidx"]
                nc.sync.dma_start(out=dbg_ps_e[:, t - 1, b_ * BLK:(b_ + 1) * BLK], in_=pscp)
            s23n = s23buf.tile([NT23, BLK], f32r, tag="s23")
            nc.scalar.activation(s23n, ps23[0:NT23, :], AF.Sign,
                                 bias=nthr23[:, t - 1:t], scale=1.0)
            if DEBUG_DUMP:
                b_ = st["bidx"]
                nc.sync.dma_start(out=dbg_s23_e[:, t - 1, b_ * BLK:(b_ + 1) * BLK], in_=s23n.bitcast(f32))
            st["s23"] = s23n

        def block_flush(st, b):
            s23, ps23 = st["s23"], st["ps23"]
            # t=17: feed [sig2_16; sig3_15]
            nc.tensor.matmul(ps23, lhsB, s23, start=False, stop=False,
                             skip_group_check=True)
            nc.tensor.matmul(ps23, lhsBlo, s23, start=False, stop=False,
                             skip_group_check=True)
            s23f = s23buf.tile([NT23, BLK], f32r, tag="s23")
            nc.scalar.activation(s23f, ps23[0:NT23, :], AF.Sign,
                                 bias=nthr23[:, DUR:DUR + 1], scale=1.0)
            if DEBUG_DUMP:
                nc.sync.dma_start(out=dbg_s23_e[:, DUR, b * BLK:(b + 1) * BLK], in_=s23f.bitcast(f32))
            # t=18: counts flush with sig3_16 (sig2-part junk is harmless)
            nc.tensor.matmul(ps23, lhsB, s23f, start=False, stop=True,
                             skip_group_check=True)
            ot = obuf.tile([2 * OUT, BLK], f32, tag="ot")
            nc.scalar.activation(ot, ps23[H2:MOUT, :], AF.Identity,
                                 bias=obi[:, 0:1], scale=osc[:, 0:1])
            nc.sync.dma_start(out=out_e[:, b * BLK:(b + 1) * BLK], in_=ot[OUT:2 * OUT, :])

        # process blocks in interleaved pairs to keep engines busy across the
        # serial per-block timestep recurrence
        for bp in range(NBLK // 2):
            a, b = 2 * bp, 2 * bp + 1
            sta = block_setup(a)
            stb = block_setup(b)
            for t in range(1, DUR + 1):
                block_step(sta, t)
                block_step(stb, t)
            block_flush(sta, a)
            block_flush(stb, b)

    nc.finalize()
    return nc


_CACHE = {}


def _get_nc():
    if "nc" not in _CACHE:
        _CACHE["nc"] = _build_program()
    return _CACHE["nc"]


def _prep_inputs(features, W1, b1, W2, b2, W3, b3, out_scale):
    """Host-side layout/constant-folding. Returns list of per-core in_maps."""
    f = np.ascontiguousarray(np.asarray(features, np.float32))
    W1 = np.asarray(W1, np.float32); b1 = np.asarray(b1, np.float32)
    W2 = np.asarray(W2, np.float32); b2 = np.asarray(b2, np.float32)
    W3 = np.asarray(W3, np.float32); b3 = np.asarray(b3, np.float32)
    out_scale = np.asarray(out_scale, np.float32)

    # features.T arranged [PCH, NCH, B]
    ftA = np.ascontiguousarray(f.T.reshape(NCH, PCH, B).transpose(1, 0, 2))

    W1r = fp32r_round(W1)
    dW1 = fp32r_round((W1r.astype(np.float64) - W1.astype(np.float64)).astype(np.float32))
    # [784,128] -> [NCH, PCH, H1] -> [PCH, NCH, H1]
    w1t = np.ascontiguousarray(W1r.T.reshape(NCH, PCH, H1).transpose(1, 0, 2))
    dw1t = np.ascontiguousarray(dW1.T.reshape(NCH, PCH, H1).transpose(1, 0, 2))

    nhi = fp32r_round((-0.5 * np.eye(H1)).astype(np.float32))
    itneg = np.zeros((H1, DUR, H1), np.float32)
    for j in range(DUR):
        itneg[:, j, :] = -(j + 1) * np.eye(H1, dtype=np.float32)
    itneg = fp32r_round(itneg)

    A_hi = fp32r_round((W2.T / 2.0).astype(np.float32))
    A_lo = fp32r_round(((W2.T / 2.0).astype(np.float64) - A_hi.astype(np.float64)).astype(np.float32))
    W3h = fp32r_round((W3.T / 2.0).astype(np.float32))
    W3l = fp32r_round(((W3.T / 2.0).astype(np.float64) - W3h.astype(np.float64)).astype(np.float32))
    lhsA = np.zeros((H1, MOUT), np.float32); lhsA[:, :H2] = A_hi
    lhsAlo = np.zeros((H1, MOUT), np.float32); lhsAlo[:, :H2] = A_lo
    lhsB = np.zeros((NT23, MOUT), np.float32)
    lhsB[:H2, :H2] = -0.5 * np.eye(H2, dtype=np.float32)
    lhsB[:H2, H2:H2 + OUT] = W3h
    lhsB[H2:, H2:H2 + OUT] = -0.5 * np.eye(OUT, dtype=np.float32)
    lhsB[H2:, H2 + OUT:] = 0.5 * np.eye(OUT, dtype=np.float32)
    lhsBlo = np.zeros((NT23, MOUT), np.float32)
    lhsBlo[:H2, H2:H2 + OUT] = W3l

    # per-partition negative thresholds (fp64 folds, consistent with rounded weights)
    rs2 = 2.0 * (A_hi.astype(np.float64) + A_lo.astype(np.float64)).sum(axis=0)
    rs3 = 2.0 * (W3h.astype(np.float64) + W3l.astype(np.float64)).sum(axis=0)
    b1d, b2d, b3d = b1.astype(np.float64), b2.astype(np.float64), b3.astype(np.float64)
    nthr1 = np.zeros((H1, DUR), np.float32)
    for t in range(1, DUR + 1):
        nthr1[:, t - 1] = -(1.0 - t * b1d + (t - 1) / 2.0).astype(np.float32)
    nthr23 = np.zeros((NT23, DUR + 1), np.float32)
    for t in range(1, DUR + 2):
        if t <= DUR:
            nthr23[:H2, t - 1] = -(1.0 - t * (0.5 * rs2 + b2d) + t / 2.0).astype(np.float32)
        else:
            nthr23[:H2, t - 1] = nthr23[:H2, t - 2]
        if t >= 2:
            u = t - 1
            nthr23[H2:, t - 1] = -(2.0 - (u + 1) / 2.0 * rs3 - u * b3d + (u - 1) / 2.0).astype(np.float32)
        else:
            nthr23[H2:, t - 1] = -1e30  # force sigma3_0 = -1
    osc = np.zeros((2 * OUT, 1), np.float32)
    osc[OUT:, 0] = (out_scale.astype(np.float64) / DUR).astype(np.float32)
    obi = np.zeros((2 * OUT, 1), np.float32)
    obi[OUT:, 0] = (9.0 * out_scale.astype(np.float64) / DUR).astype(np.float32)

    shared = dict(w1t=w1t, dw1t=dw1t, nhi=nhi, itneg=itneg, lhsA=lhsA,
                  lhsAlo=lhsAlo, lhsB=lhsB, lhsBlo=lhsBlo, nthr1=nthr1,
                  nthr23=nthr23, osc=osc, obi=obi)
    in_maps = []
    for c in range(NCORES):
        sl = slice(c * B_CORE, (c + 1) * B_CORE)
        in_maps.append(dict(ft=np.ascontiguousarray(ftA[:, :, sl]), **shared))
    return in_maps


def _execute(in_maps, trace=False, **kw):
    nc = _get_nc()
    return run_bass_kernel_spmd(nc, in_maps, list(range(NCORES)), trace=trace, **kw)


def kernel(features, W1, b1, W2, b2, W3, b3, out_scale):
    in_maps = _prep_inputs(features, W1, b1, W2, b2, W3, b3, out_scale)
    res = _execute(in_maps)
    outs = [res.results[c]["outT"] for c in range(NCORES)]   # each [OUT, B_CORE]
    full = np.concatenate(outs, axis=1)                      # [OUT, B]
    return np.ascontiguousarray(full.T).astype(np.float32)   # [B, OUT]


if __name__ == "__main__":
    rng = np.random.default_rng(0)
    feats = rng.random((B, IN), np.float32)
    out = kernel(feats,
                 (rng.standard_normal((H1, IN)) * 0.05).astype(np.float32),
                 (rng.standard_normal(H1) * 0.05).astype(np.float32),
                 (rng.standard_normal((H2, H1)) * 0.1).astype(np.float32),
                 (rng.standard_normal(H2) * 0.05).astype(np.float32),
                 (rng.standard_normal((OUT, H2)) * 0.1).astype(np.float32),
                 (rng.standard_normal(OUT) * 0.05).astype(np.float32),
                 (rng.random(OUT) + 0.5).astype(np.float32))
    print(out.shape, out.dtype)
